# revision 1
# baseline (speedup 1.0000x reference)
"""TRN2 Bass kernel for nn_Augment: homography bilinear warp + gamma +
color matrix + cutout, data-parallel over 8 NeuronCores.

Self-contained: hardcodes shapes (B=32, H=W=512, C=3, 8 cores).
"""
import os
import sys
import types
import numpy as np

import concourse.bass as bass
import concourse.bacc as bacc
import concourse.mybir as mybir
import concourse.tile as tile
from concourse.bass_utils import run_bass_kernel_spmd
from concourse.tile_rust import add_dep_helper

B, H, W, CH = 32, 512, 512, 3
NCORES = 8
IPC = B // NCORES
P = 128
F32 = mybir.dt.float32
I32 = mybir.dt.int32
I16 = mybir.dt.int16

TILE_H, TILE_W = 32, 32
NTY, NTX = H // TILE_H, W // TILE_W        # 8 x 16
TPI = NTY * NTX                             # 128 tiles/image
NROUND_FULL = IPC * TPI // 8                # 64
S = TILE_H * TILE_W                         # 2048 slots/group
SW = S // 16                                # 128 wrap cols
ROWB = W * CH
IMGEL = H * ROWB

NCOEF = 34
(C_XOFF, C_YOFF_W, C_MU0, C_MU1, C_MU2, C_MV0, C_MV1, C_MV2,
 C_MD0, C_MD1, C_MD2, C_CBAND, C_IDXB, C_YOFF_D,
 C_CA, C_SA, C_CB, C_SB, C_GAMMA,
 C_M00, C_M01, C_M02, C_M10, C_M11, C_M12, C_M20, C_M21, C_M22,
 C_CLX, C_CHX, C_CLY, C_CHY) = range(32)
C_XOFF_W = 32


def _install_profhook():
    if "antenv.axon_hooks" in sys.modules:
        return
    box = [None]
    m = types.ModuleType("antenv.axon_hooks")
    m.set_axon_ntff_profile_hook = lambda h: box.__setitem__(0, h)
    m.get_axon_ntff_profile_hook = lambda: box[0]
    sys.modules["antenv.axon_hooks"] = m
    try:
        import antenv
        antenv.axon_hooks = m
        from trn_agent_boot.trn_boot import _ntff_profile_via_ctypes
        box[0] = _ntff_profile_via_ctypes("/opt/axon/libaxon_pjrt.so")
    except Exception:
        pass


def host_params(geom_u, color_u, cutout_u):
    g = geom_u.astype(np.float64)
    flip = geom_u[:, 0] > 0.5
    tilt = (g[:, 1] * 2 - 1) * (15 * np.pi / 180)
    pan = (g[:, 2] * 2 - 1) * (15 * np.pi / 180)
    rot = (g[:, 3] * 2 - 1) * (15 * np.pi / 180)
    sc = 1.0 + (g[:, 4] * 2 - 1) * 0.1
    tx = (g[:, 5] * 2 - 1) * 0.2
    ty = (g[:, 6] * 2 - 1) * 0.2
    n = geom_u.shape[0]
    Ms = np.zeros((n, 3, 3), np.float64)
    for i in range(n):
        f = -1.0 if flip[i] else 1.0
        F = np.diag([f, 1.0, 1.0])
        ct, st = np.cos(tilt[i]), np.sin(tilt[i])
        Rx = np.array([[1, 0, 0], [0, ct, -st], [0, st, ct]])
        cp, sp = np.cos(pan[i]), np.sin(pan[i])
        Ry = np.array([[cp, 0, sp], [0, 1, 0], [-sp, 0, cp]])
        cr, sr = np.cos(rot[i]), np.sin(rot[i])
        RS = np.array([[sc[i] * cr, -sc[i] * sr, 0],
                       [sc[i] * sr, sc[i] * cr, 0], [0, 0, 1]])
        T = np.array([[1, 0, tx[i]], [0, 1, ty[i]], [0, 0, 1]])
        Ms[i] = T @ RS @ Rx @ Ry @ F
    cu = color_u.astype(np.float64)
    gamma = 1.0 + (cu[:, 3] * 2 - 1) * 0.2
    theta = (cu[:, 0] * 2 - 1) * (10 * np.pi / 180)
    c, s = np.cos(theta), np.sin(theta)
    a = 1.0 / np.sqrt(3.0)
    I3 = np.eye(3)
    K = np.array([[0, -a, a], [a, 0, -a], [-a, a, 0]])
    A = np.full((3, 3), 1.0 / 3.0)
    sat = 1.0 + (cu[:, 1] * 2 - 1) * 0.4
    lum = np.array([0.299, 0.587, 0.114])
    L = np.broadcast_to(lum, (3, 3))
    bright = 1.0 + (cu[:, 2] * 2 - 1) * 0.1
    Mc = np.zeros((n, 3, 3), np.float64)
    for i in range(n):
        Rh = c[i] * I3 + s[i] * K + (1 - c[i]) * A
        Sm = sat[i] * I3 + (1 - sat[i]) * L
        Mc[i] = bright[i] * (Sm @ Rh)
    qu = cutout_u.astype(np.float64)
    capply = qu[:, 0] < 0.5
    return (Ms, Mc, gamma, capply, qu[:, 1], qu[:, 2],
            0.3 + 0.2 * qu[:, 3], 0.3 + 0.2 * qu[:, 4])


def tile_bbox(M, ty, txi):
    ys = np.array([ty * TILE_H, ty * TILE_H + TILE_H - 1], np.float64)
    xs = np.array([txi * TILE_W, txi * TILE_W + TILE_W - 1], np.float64)
    Xg, Yg = np.meshgrid(xs, ys)
    xn = Xg * (2.0 / 511.0) - 1
    yn = Yg * (2.0 / 511.0) - 1
    nu = M[0, 0] * xn + M[0, 1] * yn + M[0, 2]
    nv = M[1, 0] * xn + M[1, 1] * yn + M[1, 2]
    de = M[2, 0] * xn + M[2, 1] * yn + M[2, 2]
    px = np.clip((nu / de + 1) * 0.5 * 511, 0, 511)
    py = np.clip((nv / de + 1) * 0.5 * 511, 0, 511)
    return (int(np.floor(py.min())), int(np.floor(py.max())),
            int(np.floor(px.min())), int(np.floor(px.max())))


def build_tables(Ms, Mc, gamma, capply, ccx, ccy, ccw, cch, core, nround,
                 RB, CB):
    dmaoff = np.zeros((nround, 16), np.int32)
    coef = np.zeros((nround, P, NCOEF), np.float32)
    for r in range(nround):
        for g in range(8):
            t = r * 8 + g
            b = t // TPI
            tl = t % TPI
            ty, txi = tl // NTX, tl % NTX
            bi = core * IPC + b
            y0, y1, x0, x1 = tile_bbox(Ms[bi], ty, txi)
            Y0 = max(0, min(y0 - 1, 511 - RB))
            X0 = max(0, min(x0 - 1, 511 - CB))
            M = Ms[bi].astype(np.float32)
            base = b * IMGEL
            dmaoff[r, 2 * g + 0] = base + Y0 * ROWB + X0 * CH
            dmaoff[r, 2 * g + 1] = base + min(Y0 + 1, H - 1) * ROWB + X0 * CH
            TX0, TY0 = txi * TILE_W, ty * TILE_H
            for L in range(16):
                p = 16 * g + L
                cf = coef[r, p]
                cf[C_XOFF] = np.float32(TX0)
                cf[C_XOFF_W] = np.float32(TX0 - 16 * g)
                cf[C_YOFF_W] = np.float32(TY0)
                cf[C_YOFF_D] = np.float32(TY0 - 32 * g)
                for j, v in zip((C_MU0, C_MU1, C_MU2, C_MV0, C_MV1, C_MV2,
                                 C_MD0, C_MD1, C_MD2),
                                M.reshape(-1)):
                    cf[j] = v
                cf[C_CBAND] = np.float32(CB)
                cf[C_IDXB] = np.float32(-(Y0 * CB + X0))
                dy, dx = (L >> 1) & 1, L & 1
                cf[C_CA] = np.float32(1 - dx)
                cf[C_SA] = np.float32(2 * dx - 1)
                cf[C_CB] = np.float32(1 - dy)
                cf[C_SB] = np.float32(2 * dy - 1)
                cf[C_GAMMA] = np.float32(gamma[bi])
                cf[C_M00:C_M00 + 9] = Mc[bi].reshape(-1).astype(np.float32)
                if capply[bi]:
                    lx = 511.0 * (ccx[bi] - ccw[bi] * 0.5) - TX0
                    hx = 511.0 * (ccx[bi] + ccw[bi] * 0.5) - TX0
                    ly = 511.0 * (ccy[bi] - cch[bi] * 0.5) - TY0
                    hy = 511.0 * (ccy[bi] + cch[bi] * 0.5) - TY0
                    cf[C_CLX] = np.float32(np.floor(lx) + 0.5)
                    cf[C_CHX] = np.float32(np.ceil(hx) - 0.5)
                    cf[C_CLY] = np.float32(np.floor(ly) + 0.5)
                    cf[C_CHY] = np.float32(np.ceil(hy) - 0.5)
                else:
                    cf[C_CLX] = cf[C_CLY] = np.float32(-9.5)
                    cf[C_CHX] = cf[C_CHY] = np.float32(-8.5)
    return dmaoff, coef


def global_band_shape(Ms):
    RB = CB = 0
    for bi in range(Ms.shape[0]):
        for ty in range(NTY):
            for txi in range(NTX):
                y0, y1, x0, x1 = tile_bbox(Ms[bi], ty, txi)
                RB = max(RB, y1 - y0 + 3)
                CB = max(CB, x1 - x0 + 3)
    assert RB * CB <= 10920, f"band too big: {RB}x{CB}"
    return RB, CB


def build_kernel(RB, CB, nround, sim_floor=False):
    nc = bacc.Bacc("TRN2", target_bir_lowering=False, debug=False,
                   enable_asserts=False, num_devices=NCORES)
    img = nc.dram_tensor("img", [IPC * IMGEL], F32, kind="ExternalInput")
    dmaoff_d = nc.dram_tensor("dmaoff", [nround, 16], I32,
                              kind="ExternalInput")
    coef_d = nc.dram_tensor("coef", [nround, P, NCOEF], F32,
                            kind="ExternalInput")
    out_d = nc.dram_tensor("out", [IPC * IMGEL], F32, kind="ExternalOutput")
    BAND = RB * CB
    AX = mybir.AluOpType
    AF = mybir.ActivationFunctionType

    with tile.TileContext(nc) as tc:
        with tc.tile_pool(name="cpool", bufs=1) as cpool, \
             tc.tile_pool(name="bpool", bufs=1) as bpool, \
             tc.tile_pool(name="wpool", bufs=2) as wpool:
            # ---- persistent iotas ----
            ii = cpool.tile([P, S], I32, name="ii")
            xw = cpool.tile([P, SW], F32, name="xw")
            yw = cpool.tile([P, SW], F32, name="yw")
            xs_c = cpool.tile([P, S], F32, name="xs_c")
            ys_c = cpool.tile([P, S], F32, name="ys_c")
            xd_c = cpool.tile([P, 3 * S // 16], F32, name="xd_c")
            yd_c = cpool.tile([P, 3 * S // 16], F32, name="yd_c")
            # wrap: free k=(k2:SW/2, kp:2): x_raw = p + 16*kp ; y = k2
            nc.gpsimd.iota(ii[:, :SW], [[0, SW // 2], [16, 2]],
                           channel_multiplier=1)
            nc.vector.tensor_copy(xw[:], ii[:, :SW])
            nc.gpsimd.iota(ii[:, :SW], [[1, SW // 2], [0, 2]],
                           channel_multiplier=0)
            nc.vector.tensor_copy(yw[:], ii[:, :SW])
            # stream: i = s: x = s%TILE_W, y = s//TILE_W
            nc.gpsimd.iota(ii[:], [[0, S // TILE_W], [1, TILE_W]],
                           channel_multiplier=0)
            nc.vector.tensor_copy(xs_c[:], ii[:])
            nc.gpsimd.iota(ii[:], [[1, S // TILE_W], [0, TILE_W]],
                           channel_multiplier=0)
            nc.vector.tensor_copy(ys_c[:], ii[:])
            # dense: free f=(j1:4, j2:32, c:3): x = j2 ;
            # y_raw = 4p + j1 (fold -64g + TY0 via C_YOFF_D)
            nd = 3 * S // 16
            nc.gpsimd.iota(ii[:, :nd], [[0, S // 16 // 32], [1, 32], [0, 3]],
                           channel_multiplier=0)
            nc.vector.tensor_copy(xd_c[:], ii[:, :nd])
            nc.gpsimd.iota(ii[:, :nd], [[1, S // 16 // 32], [0, 32], [0, 3]],
                           channel_multiplier=2)
            nc.vector.tensor_copy(yd_c[:], ii[:, :nd])

            boffreg = nc.alloc_register(mybir.EngineType.SP, "boffreg")
            boffreg_a = nc.alloc_register(mybir.EngineType.Activation,
                                          "boffrega")
            for r in range(nround):
                cf = wpool.tile([P, NCOEF], F32, tag="cf")
                bo = wpool.tile([1, 16], I32, tag="bo")
                nc.sync.dma_start(cf[:], coef_d[r:r + 1, :, :].rearrange(
                    "o p c -> (o p) c"))
                i_bo = nc.sync.dma_start(bo[:], dmaoff_d[r:r + 1, :])

                def sc(i):
                    return cf[:, i:i + 1]

                # ---- band loads ----
                band = bpool.tile([P, BAND * CH], F32, tag="band")
                if sim_floor:
                    nc.gpsimd.memset(band[:], 0.0)
                for g in range(8):
                    for half in range(2):
                        eng = nc.sync if (g & 1) == 0 else nc.scalar
                        regh = boffreg if (g & 1) == 0 else boffreg_a
                        i_rl = nc.reg_load(
                            regh, bo[0:1, 2 * g + half:2 * g + half + 1])
                        add_dep_helper(i_rl.ins, i_bo.ins, sync=True,
                                       reason="boff raw")
                        src = bass.AP(img, regh,
                                      [[3, 2], [ROWB, RB], [1, CB * CH]])
                        dst = band[16 * g + 2 * half:16 * g + 2 * half + 2, :]
                        eng.dma_start(
                            dst.rearrange("p (r c) -> p r c", r=RB), src)

                # ---- wrap pipe -> gather idx ----
                s_a = wpool.tile([P, S], F32, tag="s_a")
                s_b = wpool.tile([P, S], F32, tag="s_b")
                s_c = wpool.tile([P, S], F32, tag="s_c")
                s_d = wpool.tile([P, S], F32, tag="s_d")
                s_e = wpool.tile([P, S], F32, tag="s_e")
                s_f = wpool.tile([P, S], F32, tag="s_f")
                s_i = wpool.tile([P, S], I32, tag="s_i")
                idx16 = wpool.tile([P, SW], I16, tag="idx16")

                def geom(xr, yr, n, yoff_c, xoff_c):
                    # s_a = xn, s_b = yn
                    nc.vector.tensor_scalar(
                        s_a[:, :n], xr, sc(xoff_c), None, AX.add)
                    nc.vector.tensor_scalar(
                        s_a[:, :n], s_a[:, :n], float(2.0 / 511.0), -1.0,
                        AX.mult, AX.add)
                    nc.vector.tensor_scalar(
                        s_b[:, :n], yr, sc(yoff_c), None, AX.add)
                    nc.vector.tensor_scalar(
                        s_b[:, :n], s_b[:, :n], float(2.0 / 511.0), -1.0,
                        AX.mult, AX.add)
                    # s_c = nu, s_d = nv, s_e = den
                    for dst, c0, c1, c2 in ((s_c, C_MU0, C_MU1, C_MU2),
                                            (s_d, C_MV0, C_MV1, C_MV2),
                                            (s_e, C_MD0, C_MD1, C_MD2)):
                        nc.vector.tensor_scalar(
                            s_f[:, :n], s_b[:, :n], sc(c1), sc(c2),
                            AX.mult, AX.add)
                        nc.vector.scalar_tensor_tensor(
                            dst[:, :n], s_a[:, :n], sc(c0), s_f[:, :n],
                            AX.mult, AX.add)
                    nc.vector.reciprocal(s_e[:, :n], s_e[:, :n])
                    # px = (nu*rde + 1)*255.5 (matches reference order mostly)
                    for dst in (s_c, s_d):
                        nc.vector.tensor_tensor(dst[:, :n], dst[:, :n],
                                                s_e[:, :n], AX.mult)
                        nc.vector.tensor_scalar(dst[:, :n], dst[:, :n],
                                                1.0, 255.5, AX.add, AX.mult)
                    # clipped: s_a = pxc, s_b = pyc
                    nc.vector.tensor_scalar(s_a[:, :n], s_c[:, :n], 0.0,
                                            511.0, AX.max, AX.min)
                    nc.vector.tensor_scalar(s_b[:, :n], s_d[:, :n], 0.0,
                                            511.0, AX.max, AX.min)
                    # floors: s_e = x0f, s_f = y0f
                    for src_t, dst_t in ((s_a, s_e), (s_b, s_f)):
                        if sim_floor:
                            nc.vector.tensor_copy(s_i[:, :n], src_t[:, :n])
                        else:
                            nc.vector.tensor_scalar(
                                dst_t[:, :n], src_t[:, :n], 1.0, -0.5,
                                AX.mult, AX.add)
                            nc.vector.tensor_copy(s_i[:, :n], dst_t[:, :n])
                        nc.vector.tensor_copy(dst_t[:, :n], s_i[:, :n])
                        nc.vector.tensor_scalar(dst_t[:, :n], dst_t[:, :n],
                                                510.0, 0.0, AX.min, AX.max)

                geom(xw[:], yw[:], SW, C_YOFF_W, C_XOFF_W)
                # idx = y0f*CB + x0f + bias -> i16
                nc.vector.scalar_tensor_tensor(
                    s_f[:, :SW], s_f[:, :SW], sc(C_CBAND), s_e[:, :SW],
                    AX.mult, AX.add)
                nc.vector.tensor_scalar(
                    s_f[:, :SW], s_f[:, :SW], sc(C_IDXB), None, AX.add)
                nc.vector.tensor_copy(s_i[:, :SW], s_f[:, :SW])
                nc.vector.tensor_copy(idx16[:], s_i[:, :SW])

                # ---- gather ----
                gout = bpool.tile([P, S * CH], F32, tag="gout")
                i_gth = nc.gpsimd.ap_gather(
                    gout[:].rearrange("p (n d) -> p n d", d=CH),
                    band[:].rearrange("p (n d) -> p n d", d=CH),
                    idx16[:], channels=P, num_elems=BAND, d=CH, num_idxs=S)

                # ---- weights (stream layout) ----
                geom(xs_c[:], ys_c[:], S, C_YOFF_W, C_XOFF)
                # valid = eq(px, pxc)*eq(py, pyc)
                nc.vector.tensor_tensor(s_c[:], s_c[:], s_a[:], AX.is_equal)
                nc.vector.tensor_tensor(s_d[:], s_d[:], s_b[:], AX.is_equal)
                nc.vector.tensor_tensor(s_c[:], s_c[:], s_d[:], AX.mult)
                # wx = pxc - x0f -> s_a ; wy -> s_b
                nc.vector.tensor_tensor(s_a[:], s_a[:], s_e[:], AX.subtract)
                nc.vector.tensor_tensor(s_b[:], s_b[:], s_f[:], AX.subtract)
                # w = (CA + SA*wx) * (CB + SB*wy) * valid  -> s_d
                nc.vector.tensor_scalar(s_e[:], s_a[:], sc(C_SA), sc(C_CA),
                                        AX.mult, AX.add)
                nc.vector.tensor_scalar(s_f[:], s_b[:], sc(C_SB), sc(C_CB),
                                        AX.mult, AX.add)
                nc.vector.tensor_tensor(s_d[:], s_e[:], s_f[:], AX.mult)
                i_wd = nc.vector.tensor_tensor(s_d[:], s_d[:], s_c[:],
                                               AX.mult)
                # apply weights: gout *= w (broadcast over ch)
                wb = bass.AP(s_d.tensor, s_d[:].offset,
                             [s_d[:].ap[0], [1, S], [0, CH]])
                i_wm = nc.vector.tensor_tensor(
                    gout[:].rearrange("p (n d) -> p n d", d=CH),
                    gout[:].rearrange("p (n d) -> p n d", d=CH),
                    wb, AX.mult)
                add_dep_helper(i_wm.ins, i_gth.ins, sync=True, reason="graw")
                add_dep_helper(i_wm.ins, i_wd.ins, sync=True, reason="wraw")

                # ---- combine across shift lanes ----
                # partition shifts via DMA (DVE can't read partition!=0 start)
                acc = bpool.tile([P, S * CH], F32, tag="acc")
                i_s1 = nc.sync.dma_start(acc[0:127, :], gout[1:128, :])
                add_dep_helper(i_s1.ins, i_wm.ins, sync=True, reason="s1")
                i_a1 = nc.vector.tensor_tensor(
                    gout[0:127, :], gout[0:127, :], acc[0:127, :], AX.add)
                add_dep_helper(i_a1.ins, i_s1.ins, sync=True, reason="a1")
                i_s2 = nc.sync.dma_start(acc[0:126, :], gout[2:128, :])
                add_dep_helper(i_s2.ins, i_a1.ins, sync=True, reason="s2")
                i_c2 = nc.vector.tensor_tensor(
                    gout[0:126, :], gout[0:126, :], acc[0:126, :], AX.add)
                add_dep_helper(i_c2.ins, i_s2.ins, sync=True, reason="c2")

                # ---- repack to dense ----
                dense = wpool.tile([P, 3 * S // 16], F32, tag="dense")
                i_rp = nc.sync.dma_start(dense[:], gout[0:128:16, :])
                add_dep_helper(i_rp.ins, i_c2.ins, sync=True, reason="rp")

                # ---- post pipe (dense layout [P, 384]) ----
                d_t = wpool.tile([P, 3 * S // 16], F32, tag="d_t")
                d_u = wpool.tile([P, 3 * S // 16], F32, tag="d_u")
                i_pp = nc.vector.tensor_scalar(dense[:], dense[:], 1e-30,
                                               1.0, AX.max, AX.min)
                add_dep_helper(i_pp.ins, i_rp.ins, sync=True, reason="pp")
                nc.scalar.activation(dense[:], dense[:], AF.Ln)
                nc.scalar.activation(dense[:], dense[:], AF.Exp,
                                     scale=cf[:, C_GAMMA:C_GAMMA + 1])
                # color: d_t[c] = sum_k Mc[c,k] * dense[k]
                nch = S // 16
                nd = 3 * S // 16
                dv = dense[:].rearrange("p (n c) -> p n c", c=CH)
                tv = d_t[:].rearrange("p (n c) -> p n c", c=CH)
                uv = d_u[:, :nch].rearrange("p (n o) -> p n o", o=1)
                for c3 in range(3):
                    nc.vector.tensor_scalar(
                        tv[:, :, c3:c3 + 1], dv[:, :, 0:1],
                        sc(C_M00 + 3 * c3), None, AX.mult)
                    nc.vector.tensor_scalar(
                        uv, dv[:, :, 1:2], sc(C_M00 + 3 * c3 + 1), None,
                        AX.mult)
                    nc.vector.tensor_tensor(
                        tv[:, :, c3:c3 + 1], tv[:, :, c3:c3 + 1], uv, AX.add)
                    nc.vector.tensor_scalar(
                        uv, dv[:, :, 2:3], sc(C_M00 + 3 * c3 + 2), None,
                        AX.mult)
                    nc.vector.tensor_tensor(
                        tv[:, :, c3:c3 + 1], tv[:, :, c3:c3 + 1], uv, AX.add)
                nc.vector.tensor_scalar(d_t[:], d_t[:], 0.0, 1.0,
                                        AX.max, AX.min)
                # cutout: mask=1 inside box -> zero
                nc.vector.tensor_scalar(d_u[:], xd_c[:], sc(C_CLX), sc(C_CHX),
                                        AX.max, AX.min)
                nc.vector.tensor_tensor(d_u[:], d_u[:], xd_c[:], AX.is_equal)
                du2 = wpool.tile([P, 3 * S // 16], F32, tag="du2")
                nc.vector.tensor_scalar(du2[:], yd_c[:], sc(C_YOFF_D), None,
                                        AX.add)
                nc.vector.tensor_scalar(du2[:], du2[:], sc(C_YOFF_W), None,
                                        AX.subtract)
                du3 = wpool.tile([P, 3 * S // 16], F32, tag="du3")
                nc.vector.tensor_scalar(du3[:], du2[:], sc(C_CLY), sc(C_CHY),
                                        AX.max, AX.min)
                nc.vector.tensor_tensor(du3[:], du3[:], du2[:], AX.is_equal)
                nc.vector.tensor_tensor(d_u[:], d_u[:], du3[:], AX.mult)
                nc.vector.tensor_scalar(d_u[:], d_u[:], -1.0, 1.0,
                                        AX.mult, AX.add)
                nc.vector.tensor_tensor(d_t[:], d_t[:], d_u[:], AX.mult)

                # ---- output stores ----
                t8 = r * 8
                b = t8 // TPI
                tl = t8 % TPI
                ty, tx0 = tl // NTX, tl % NTX
                nrows_l = S // 16 // 32
                for r2 in range(nrows_l):
                    off = b * IMGEL + (ty * TILE_H + r2) * ROWB + tx0 * 96
                    dst = bass.AP(out_d, off,
                                  [[96, 8], [nrows_l * ROWB, 16], [1, 96]])
                    nc.sync.dma_start(
                        dst, d_t[:, 96 * r2:96 * r2 + 96])
    nc.compile()
    return nc


_CACHE = {}


def kernel(images, geom_u, color_u, cutout_u):
    _install_profhook()
    images = np.asarray(images, dtype=np.float32)
    geom_u = np.asarray(geom_u, dtype=np.float32)
    color_u = np.asarray(color_u, dtype=np.float32)
    cutout_u = np.asarray(cutout_u, dtype=np.float32)
    Ms, Mc, gamma, capply, ccx, ccy, ccw, cch = host_params(
        geom_u, color_u, cutout_u)
    RB, CB = global_band_shape(Ms)
    nround = int(os.environ.get('KROUNDS', NROUND_FULL))
    sim_floor = os.environ.get("KSIM", "0") == "1"
    key = (RB, CB, nround, sim_floor)
    if key not in _CACHE:
        _CACHE[key] = build_kernel(RB, CB, nround, sim_floor=sim_floor)
    nc = _CACHE[key]
    in_maps = []
    for core in range(NCORES):
        dmaoff, coef = build_tables(Ms, Mc, gamma, capply, ccx, ccy, ccw,
                                    cch, core, nround, RB, CB)
        shard = images[core * IPC:(core + 1) * IPC].reshape(-1)
        in_maps.append({"img": np.ascontiguousarray(shard),
                        "dmaoff": dmaoff, "coef": coef})
    if sim_floor:
        from concourse.bass_interp import CoreSim
        sim = CoreSim(nc, trace=False)
        for k, v in in_maps[0].items():
            sim.tensor(k)[:] = v
        sim.simulate()
        out = np.zeros((B, H, W, CH), np.float32)
        out[0:IPC] = np.array(sim.tensor("out")).reshape(IPC, H, W, CH)
        return out
    trace = os.environ.get("KTRACE", "0") == "1"
    res = run_bass_kernel_spmd(nc, in_maps, core_ids=list(range(NCORES)),
                               trace=trace, trace_cores=[0] if trace else None)
    if trace and res.exec_time_ns:
        print(f"HW exec time: {res.exec_time_ns} ns")
    out = np.zeros((B, H, W, CH), np.float32)
    for core in range(NCORES):
        out[core * IPC:(core + 1) * IPC] = (
            res.results[core]["out"].reshape(IPC, H, W, CH))
    return out



# revision 21
# speedup vs baseline: 1.5486x; 1.5486x over previous
"""TRN2 Bass kernel for nn_Augment: homography bilinear warp + gamma +
color matrix + cutout, data-parallel over 8 NeuronCores.

v4 architecture:
- 16w x 32h output tiles (512 px). Each Q7 core (16 partitions) handles 4
  tiles/round via 4 tap-partitions per tile. 32 tiles per NeuronCore round
  (one full 32-row strip of one image), 64 rounds.
- Geometry computed densely in two wrap layouts [128,128] (no replication):
  layout A (gather index order) for idx, layout B (transpose order) for the
  4 bilinear tap weights (+valid +cutout folded in).
- Tap weights transported to gather-output alignment with a single DVE
  32x32 stream transpose.
- Band: one HBM DMA per tile loads 2 x-shifted copies ((RB+1) rows); two
  static SBUF DMAs per round replicate row-shifted copies for the dy taps.
- Bilinear combine via TensorE matmuls into PSUM (4 accumulation phases
  select each tile's 4 tap partitions), then one DMA repack to a dense
  [128, 384] layout for gamma (ScalarE) + color matrix + clips.

Self-contained: hardcodes shapes (B=32, H=W=512, C=3, 8 cores).
"""
import os
import sys
import types
import numpy as np

import concourse.bass as bass
import concourse.bacc as bacc
import concourse.mybir as mybir
import concourse.tile as tile
from concourse.bass_utils import run_bass_kernel_spmd
from concourse.tile_rust import add_dep_helper

B, H, W, CH = 32, 512, 512, 3
NCORES = 8
IPC = B // NCORES
P = 128
F32 = mybir.dt.float32
I32 = mybir.dt.int32
I16 = mybir.dt.int16

TILE_H, TILE_W = 32, 16
NTY, NTX = H // TILE_H, W // TILE_W          # 16 x 32
TPI = NTY * NTX                              # 512 tiles/image
TPR = 32                                     # tiles per round (one strip)
NROUND_FULL = IPC * TPI // TPR               # 64
S = TILE_H * TILE_W                          # 512 px/tile
POS = 4 * S                                  # 2048 positions per Q7 core
ROWB = W * CH
IMGEL = H * ROWB
SCL = 2.0 / 511.0

# ---- coef column layout ----
# layout A (idx): slopes (per-partition scalars) + per-quarter intercepts
A_SU, A_SV, A_SD = 0, 1, 2
A_BU, A_BV, A_BD, A_IDXB = 3, 7, 11, 15     # 4 cols each
# layout B (weights): slopes + per-(h,q) intercepts (8 segs)
B_SU, B_SV, B_SD = 19, 20, 21
B_BU, B_BV, B_BD, B_CLY, B_CHY = 22, 30, 38, 46, 54   # 8 cols each
# dense layout: gamma + color matrix
D_G = 62
D_M = 63                                     # 9 cols
NCOEF = 72


def _install_profhook():
    if "antenv.axon_hooks" in sys.modules:
        return
    box = [None]
    m = types.ModuleType("antenv.axon_hooks")
    m.set_axon_ntff_profile_hook = lambda h: box.__setitem__(0, h)
    m.get_axon_ntff_profile_hook = lambda: box[0]
    sys.modules["antenv.axon_hooks"] = m
    try:
        import antenv
        antenv.axon_hooks = m
        from trn_agent_boot.trn_boot import _ntff_profile_via_ctypes
        box[0] = _ntff_profile_via_ctypes("/opt/axon/libaxon_pjrt.so")
    except Exception:
        pass


def host_params(geom_u, color_u, cutout_u):
    g = geom_u.astype(np.float64)
    flip = geom_u[:, 0] > 0.5
    tilt = (g[:, 1] * 2 - 1) * (15 * np.pi / 180)
    pan = (g[:, 2] * 2 - 1) * (15 * np.pi / 180)
    rot = (g[:, 3] * 2 - 1) * (15 * np.pi / 180)
    sc = 1.0 + (g[:, 4] * 2 - 1) * 0.1
    tx = (g[:, 5] * 2 - 1) * 0.2
    ty = (g[:, 6] * 2 - 1) * 0.2
    n = geom_u.shape[0]
    Ms = np.zeros((n, 3, 3), np.float64)
    for i in range(n):
        f = -1.0 if flip[i] else 1.0
        F = np.diag([f, 1.0, 1.0])
        ct, st = np.cos(tilt[i]), np.sin(tilt[i])
        Rx = np.array([[1, 0, 0], [0, ct, -st], [0, st, ct]])
        cp, sp = np.cos(pan[i]), np.sin(pan[i])
        Ry = np.array([[cp, 0, sp], [0, 1, 0], [-sp, 0, cp]])
        cr, sr = np.cos(rot[i]), np.sin(rot[i])
        RS = np.array([[sc[i] * cr, -sc[i] * sr, 0],
                       [sc[i] * sr, sc[i] * cr, 0], [0, 0, 1]])
        T = np.array([[1, 0, tx[i]], [0, 1, ty[i]], [0, 0, 1]])
        Ms[i] = T @ RS @ Rx @ Ry @ F
    cu = color_u.astype(np.float64)
    gamma = 1.0 + (cu[:, 3] * 2 - 1) * 0.2
    theta = (cu[:, 0] * 2 - 1) * (10 * np.pi / 180)
    c, s = np.cos(theta), np.sin(theta)
    a = 1.0 / np.sqrt(3.0)
    I3 = np.eye(3)
    K = np.array([[0, -a, a], [a, 0, -a], [-a, a, 0]])
    A = np.full((3, 3), 1.0 / 3.0)
    sat = 1.0 + (cu[:, 1] * 2 - 1) * 0.4
    lum = np.array([0.299, 0.587, 0.114])
    L = np.broadcast_to(lum, (3, 3))
    bright = 1.0 + (cu[:, 2] * 2 - 1) * 0.1
    Mc = np.zeros((n, 3, 3), np.float64)
    for i in range(n):
        Rh = c[i] * I3 + s[i] * K + (1 - c[i]) * A
        Sm = sat[i] * I3 + (1 - sat[i]) * L
        Mc[i] = bright[i] * (Sm @ Rh)
    qu = cutout_u.astype(np.float64)
    capply = qu[:, 0] < 0.5
    return (Ms, Mc, gamma, capply, qu[:, 1], qu[:, 2],
            0.3 + 0.2 * qu[:, 3], 0.3 + 0.2 * qu[:, 4])


def tile_bbox(M, ty, txi):
    ys = np.array([ty * TILE_H, ty * TILE_H + TILE_H - 1], np.float64)
    xs = np.array([txi * TILE_W, txi * TILE_W + TILE_W - 1], np.float64)
    Xg, Yg = np.meshgrid(xs, ys)
    xn = Xg * SCL - 1
    yn = Yg * SCL - 1
    nu = M[0, 0] * xn + M[0, 1] * yn + M[0, 2]
    nv = M[1, 0] * xn + M[1, 1] * yn + M[1, 2]
    de = M[2, 0] * xn + M[2, 1] * yn + M[2, 2]
    px = np.clip((nu / de + 1) * 0.5 * 511, 0, 511)
    py = np.clip((nv / de + 1) * 0.5 * 511, 0, 511)
    return (int(np.floor(py.min())), int(np.floor(py.max())),
            int(np.floor(px.min())), int(np.floor(px.max())))


def global_band_shape(Ms):
    RB = CB = 0
    for bi in range(Ms.shape[0]):
        for ty in range(NTY):
            for txi in range(NTX):
                y0, y1, x0, x1 = tile_bbox(Ms[bi], ty, txi)
                RB = max(RB, y1 - y0 + 3)
                CB = max(CB, x1 - x0 + 3)
    assert (RB + 1) * CB * CH <= 2**15, f"band too big: {RB}x{CB}"
    return RB, CB


def build_tables(Ms, Mc, gamma, capply, ccx, ccy, ccw, cch, core, nround,
                 RB, CB, sim_floor):
    dmaoff = np.zeros((nround, TPR), np.int32)
    coef = np.zeros((nround, P, NCOEF), np.float64)
    pidx = np.arange(P)
    # A floor fold: on HW cast rounds-to-nearest -> px-0.5; sim cast truncs.
    FA = 255.5 if sim_floor else 255.0
    for r in range(nround):
        imgl, strip = r // 16, r % 16
        bi = core * IPC + imgl
        M = Ms[bi]
        TY0 = 32 * strip
        t0 = SCL * TY0 - 1.0
        cf = coef[r]
        # slopes (same for every partition; image constant per round)
        cf[:, A_SU] = (255.5 * M[0, 1] + FA * M[2, 1]) * SCL
        cf[:, A_SV] = (255.5 * M[1, 1] + FA * M[2, 1]) * SCL
        cf[:, A_SD] = M[2, 1] * SCL
        cf[:, B_SU] = 2 * (255.5 * M[0, 1] + 255.5 * M[2, 1]) * SCL
        cf[:, B_SV] = 2 * (255.5 * M[1, 1] + 255.5 * M[2, 1]) * SCL
        cf[:, B_SD] = 2 * M[2, 1] * SCL
        # cutout box (image coords)
        if capply[bi]:
            lx = 511.0 * (ccx[bi] - ccw[bi] * 0.5)
            hx = 511.0 * (ccx[bi] + ccw[bi] * 0.5)
            ly = 511.0 * (ccy[bi] - cch[bi] * 0.5)
            hy = 511.0 * (ccy[bi] + cch[bi] * 0.5)
        # per-tile data
        Y0s = np.zeros(TPR, np.int64)
        X0s = np.zeros(TPR, np.int64)
        for t in range(TPR):
            y0, y1, x0, x1 = tile_bbox(M, strip, t)
            Y0 = max(0, min(y0 - 1, H - 1 - RB))
            X0 = max(0, min(x0 - 1, W - 1 - CB))
            Y0s[t], X0s[t] = Y0, X0
            dmaoff[r, t] = imgl * IMGEL + Y0 * ROWB + X0 * CH
        # ---- layout A (g = p//16, m = p%16, quarter q -> tile 4g+q) ----
        for q in range(4):
            tt = (pidx // 16) * 4 + q           # tile per partition
            TX0 = 16 * tt
            xn = SCL * (TX0 + (pidx % 16)) - 1.0
            nui = M[0, 0] * xn + M[0, 1] * t0 + M[0, 2]
            nvi = M[1, 0] * xn + M[1, 1] * t0 + M[1, 2]
            dei = M[2, 0] * xn + M[2, 1] * t0 + M[2, 2]
            cf[:, A_BU + q] = 255.5 * nui + FA * dei
            cf[:, A_BV + q] = 255.5 * nvi + FA * dei
            cf[:, A_BD + q] = dei
            cf[:, A_IDXB + q] = -(Y0s[tt] * CB + X0s[tt])
        # ---- layout B (G = p//32, kk = p%32, segs (h,q)) ----
        kk = pidx % 32
        xB = kk % 16
        par = kk // 16
        ypart = SCL * (TY0 + par) - 1.0
        for h in range(2):
            for q in range(4):
                tt = ((pidx // 32) * 2 + h) * 4 + q
                TX0 = 16 * tt
                xn = SCL * (TX0 + xB) - 1.0
                nui = M[0, 0] * xn + M[0, 1] * ypart + M[0, 2]
                nvi = M[1, 0] * xn + M[1, 1] * ypart + M[1, 2]
                dei = M[2, 0] * xn + M[2, 1] * ypart + M[2, 2]
                sg = 4 * h + q
                cf[:, B_BU + sg] = 255.5 * (nui + dei)
                cf[:, B_BV + sg] = 255.5 * (nvi + dei)
                cf[:, B_BD + sg] = dei
                if capply[bi]:
                    xg = TX0 + xB
                    xin = (xg >= np.floor(lx) + 1) & (xg <= np.ceil(hx) - 1)
                    cly = np.where(xin, np.floor(ly) + 0.5 - TY0, 5000.0)
                    chy = np.where(xin, np.ceil(hy) - 0.5 - TY0, -5000.0)
                else:
                    cly, chy = 5000.0, -5000.0
                cf[:, B_CLY + sg] = cly
                cf[:, B_CHY + sg] = chy
        # ---- dense (tile = p//4) ----
        cf[:, D_G] = gamma[bi]
        cf[:, D_M:D_M + 9] = Mc[bi].reshape(-1)
    return dmaoff, coef.astype(np.float32)


def build_wsel():
    # phase (b, q): out partition 32*b + tile (tile = 4g+q) sums the 4 tap
    # partitions 16g + 4q + t; moving slice is gout[:, 1536q + 384b : +384].
    ws = np.zeros((16, P, P), np.float32)
    for bq in range(16):
        b, q = bq // 4, bq % 4
        for g in range(8):
            i = 32 * b + 4 * g + q
            for t in range(4):
                ws[bq, 16 * g + 4 * q + t, i] = 1.0
    return ws.transpose(1, 0, 2).reshape(P, 16 * P).copy()


def build_kernel(RB, CB, nround, sim_floor=False):
    nc = bacc.Bacc("TRN2", target_bir_lowering=False, debug=False,
                   enable_asserts=False, num_devices=NCORES)
    img = nc.dram_tensor("img", [IPC * IMGEL], F32, kind="ExternalInput")
    dmaoff_d = nc.dram_tensor("dmaoff", [nround, TPR], I32,
                              kind="ExternalInput")
    coef_d = nc.dram_tensor("coef", [nround, P, NCOEF], F32,
                            kind="ExternalInput")
    wsel_d = nc.dram_tensor("wsel", [P, 16 * P], F32, kind="ExternalInput")
    pconst_d = nc.dram_tensor("pconst", [P, 1], F32, kind="ExternalInput")
    out_d = nc.dram_tensor("out", [IPC * IMGEL], F32, kind="ExternalOutput")
    CB3 = CB * CH
    NELEM = (RB + 1) * CB
    AX = mybir.AluOpType
    AF = mybir.ActivationFunctionType
    FLOFF = 0.0 if sim_floor else 0.5

    with tile.TileContext(nc) as tc:
        with tc.tile_pool(name="cpool", bufs=1) as cpool, \
             tc.tile_pool(name="wpool", bufs=2) as wpool, \
             tc.tile_pool(name="bpool", bufs=2) as bpool, \
             tc.tile_pool(name="ppool", bufs=2, space="PSUM") as ppool:
            # ---- persistent constants ----
            ii = cpool.tile([P, P], I32, name="ii")
            yA = cpool.tile([P, P], F32, name="yA")
            uB = cpool.tile([P, P], F32, name="uB")
            yinB = cpool.tile([P, P], F32, name="yinB")
            parc = cpool.tile([P, 1], F32, name="parc")
            wsel = cpool.tile([P, 16 * P], F32, name="wsel")
            nc.sync.dma_start(wsel[:], wsel_d[:, :])
            # yA: value = k % 32 over cols k
            nc.gpsimd.iota(ii[:], [[0, 4], [1, 32]], channel_multiplier=0)
            nc.vector.tensor_copy(yA[:], ii[:])
            # uB: cols (h:2, q:4, u16:16) -> u16
            nc.gpsimd.iota(ii[:], [[0, 2], [0, 4], [1, 16]],
                           channel_multiplier=0)
            nc.vector.tensor_copy(uB[:], ii[:])
            # par = (p % 32) >= 16 per partition (host-provided)
            nc.sync.dma_start(parc[:], pconst_d[:, :])
            # yinB = 2*u16 + par
            nc.vector.tensor_scalar(yinB[:], uB[:], 2.0, None, AX.mult)
            nc.vector.tensor_scalar(yinB[:], yinB[:], parc[:, 0:1], None,
                                    AX.add)

            # ---- persistent w_in (unused cols stay zero/stale-finite) ----
            w_in = cpool.tile([P, POS], F32, name="w_in")
            nc.gpsimd.memset(w_in[:], 0.0)

            # registers for band-offset loads
            regs = {}
            for nm, et in (("sync", mybir.EngineType.SP),
                           ("scalar", mybir.EngineType.Activation)):
                regs[nm] = nc.alloc_register(et, f"boff_{nm}")
            BAND_ENG = (["sync"] * 16 + ["scalar"] * 16)

            for r in range(nround):
                cf = wpool.tile([P, NCOEF], F32, tag="cf")
                bo = wpool.tile([1, TPR], I32, tag="bo")
                nc.scalar.dma_start(cf[:], coef_d[r:r + 1, :, :].rearrange(
                    "o p c -> (o p) c"))
                i_bo = nc.sync.dma_start(bo[:], dmaoff_d[r:r + 1, :])

                def sc(i):
                    return cf[:, i:i + 1]

                def bcA(i):
                    b = cf[:, i:i + 4]
                    return bass.AP(b.tensor, b.offset,
                                   [b.ap[0], [1, 4], [0, 32]])

                def bcB(i):
                    b = cf[:, i:i + 8]
                    return bass.AP(b.tensor, b.offset,
                                   [b.ap[0], [1, 8], [0, 16]])

                # ---- band loads: 1 HBM DMA per tile + 2 SBUF repl ----
                band = bpool.tile([P, NELEM * CH], F32, tag="band")
                i_bands = []
                for t in range(TPR):
                    eng = BAND_ENG[t]
                    e = getattr(nc, eng)
                    i_rl = nc.reg_load(regs[eng], bo[0:1, t:t + 1])
                    add_dep_helper(i_rl.ins, i_bo.ins, sync=True,
                                   reason="boff")
                    src = bass.AP(img, regs[eng],
                                  [[3, 2], [ROWB, RB + 1], [1, CB3]])
                    g, q = t // 4, t % 4
                    p0 = 16 * g + 4 * q
                    dst = band[p0:p0 + 2, :].rearrange(
                        "p (r c) -> p r c", r=RB + 1)
                    i_bands.append(e.dma_start(dst, src))
                # dy replication: rows 1..RB -> partitions +2 (same queue
                # as the loads it reads so in-queue order applies)
                loads = list(i_bands)
                for t in range(TPR):
                    p0 = 16 * (t // 4) + 4 * (t % 4)
                    i_dy = nc.sync.dma_start(
                        band[p0 + 2:p0 + 4, 0:RB * CB3],
                        band[p0:p0 + 2, CB3:(RB + 1) * CB3])
                    for ib in loads:
                        add_dep_helper(i_dy.ins, ib.ins, sync=True,
                                       reason="dyrep")
                    i_bands.append(i_dy)

                # ---- layout A: gather indices ----
                de = wpool.tile([P, P], F32, tag="de")
                t1 = wpool.tile([P, P], F32, tag="t1")
                t2 = wpool.tile([P, P], F32, tag="t2")
                x0f = wpool.tile([P, P], F32, tag="x0f")
                y0f = wpool.tile([P, P], F32, tag="y0f")
                s_i = wpool.tile([P, P], I32, tag="s_i")
                idx16 = wpool.tile([P, P], I16, tag="idx16")
                nc.vector.scalar_tensor_tensor(
                    de[:], yA[:], sc(A_SD), bcA(A_BD), AX.mult, AX.add)
                nc.vector.reciprocal(de[:], de[:])
                for dst, scol, bcol in ((x0f, A_SU, A_BU), (y0f, A_SV, A_BV)):
                    nc.vector.scalar_tensor_tensor(
                        t1[:], yA[:], sc(scol), bcA(bcol), AX.mult, AX.add)
                    nc.vector.tensor_tensor(t1[:], t1[:], de[:], AX.mult)
                    nc.vector.tensor_copy(s_i[:], t1[:])
                    nc.vector.tensor_copy(dst[:], s_i[:])
                    nc.vector.tensor_scalar(dst[:], dst[:], 0.0, 510.0,
                                            AX.max, AX.min)
                # idx = y0f*CB + x0f + bias -> i16
                nc.vector.tensor_scalar(y0f[:], y0f[:], float(CB), None,
                                        AX.mult)
                nc.vector.tensor_tensor(y0f[:], y0f[:], x0f[:], AX.add)
                nc.vector.tensor_tensor(y0f[:], y0f[:], bcA(A_IDXB), AX.add)
                nc.vector.tensor_copy(s_i[:], y0f[:])
                nc.vector.tensor_copy(idx16[:], s_i[:])

                # ---- gather ----
                gout = bpool.tile([P, POS * CH], F32, tag="gout")
                i_gth = nc.gpsimd.ap_gather(
                    gout[:].rearrange("p (n d) -> p n d", d=CH),
                    band[:, 0:RB * CB * CH].rearrange(
                        "p (n d) -> p n d", d=CH),
                    idx16[:], channels=P, num_elems=RB * CB, d=CH,
                    num_idxs=POS)

                # ---- layout B: tap weights ----
                pxu = wpool.tile([P, P], F32, tag="pxu")
                pyu = wpool.tile([P, P], F32, tag="pyu")
                pxc = wpool.tile([P, P], F32, tag="pxc")
                pyc = wpool.tile([P, P], F32, tag="pyc")
                vv = wpool.tile([P, P], F32, tag="vv")
                nc.vector.scalar_tensor_tensor(
                    de[:], uB[:], sc(B_SD), bcB(B_BD), AX.mult, AX.add)
                nc.vector.reciprocal(de[:], de[:])
                for dst, dstc, scol, bcol in ((pxu, pxc, B_SU, B_BU),
                                              (pyu, pyc, B_SV, B_BV)):
                    nc.vector.scalar_tensor_tensor(
                        t1[:], uB[:], sc(scol), bcB(bcol), AX.mult, AX.add)
                    nc.vector.tensor_tensor(dst[:], t1[:], de[:], AX.mult)
                    nc.vector.tensor_scalar(dstc[:], dst[:], 0.0, 511.0,
                                            AX.max, AX.min)
                # valid
                nc.vector.tensor_tensor(pxu[:], pxu[:], pxc[:], AX.is_equal)
                nc.vector.tensor_tensor(pyu[:], pyu[:], pyc[:], AX.is_equal)
                nc.vector.tensor_tensor(vv[:], pxu[:], pyu[:], AX.mult)
                # cutout: ym in (CLY, CHY) -> zero weight
                nc.vector.tensor_tensor(t1[:], yinB[:], bcB(B_CLY), AX.is_ge)
                nc.vector.tensor_tensor(t2[:], yinB[:], bcB(B_CHY), AX.is_le)
                nc.vector.tensor_tensor(t1[:], t1[:], t2[:], AX.mult)
                nc.vector.tensor_scalar(t1[:], t1[:], -1.0, 1.0,
                                        AX.mult, AX.add)
                nc.vector.tensor_tensor(vv[:], vv[:], t1[:], AX.mult)
                # floors + fracs
                for pc, fl in ((pxc, x0f), (pyc, y0f)):
                    nc.vector.tensor_scalar(t1[:], pc[:], 1.0, -FLOFF,
                                            AX.mult, AX.add)
                    nc.vector.tensor_copy(s_i[:], t1[:])
                    nc.vector.tensor_copy(fl[:], s_i[:])
                    nc.vector.tensor_scalar(fl[:], fl[:], 0.0, 510.0,
                                            AX.max, AX.min)
                    nc.vector.tensor_tensor(pc[:], pc[:], fl[:], AX.subtract)
                # pxc=wx, pyc=wy now. basis: t1=A1=V*wx, t2=A0=V-A1, vv ->
                nc.vector.tensor_tensor(t1[:], vv[:], pxc[:], AX.mult)
                nc.vector.tensor_tensor(t2[:], vv[:], t1[:], AX.subtract)
                # B0 = 1-wy -> pxu (reuse)
                nc.vector.tensor_scalar(pxu[:], pyc[:], -1.0, 1.0,
                                        AX.mult, AX.add)
                # tap writes into w_in (cols 516q + 32u16 + 16h + t)
                for hh in range(2):
                    for t in range(4):
                        adx = t1 if (t & 1) else t2
                        bdy = pyc if (t & 2) else pxu
                        sb = adx[:, 64 * hh:64 * hh + 64]
                        sv = bass.AP(sb.tensor, sb.offset,
                                     [sb.ap[0], [16, 4], [1, 16]])
                        db = w_in[:, 16 * hh + t:]
                        dv = bass.AP(db.tensor, db.offset,
                                     [db.ap[0], [516, 4], [32, 16]])
                        bb = bdy[:, 64 * hh:64 * hh + 64]
                        bv = bass.AP(bb.tensor, bb.offset,
                                     [bb.ap[0], [16, 4], [1, 16]])
                        nc.vector.tensor_tensor(dv, sv, bv, AX.mult)
                # transpose to gather alignment
                w_out = wpool.tile([P, POS], F32, tag="w_out")
                nc.vector.transpose(w_out[:], w_in[:])

                # ---- apply weights ----
                wb = bass.AP(w_out.tensor, w_out[:].offset,
                             [w_out[:].ap[0], [1, POS], [0, CH]])
                i_wm = nc.vector.tensor_tensor(
                    gout[:].rearrange("p (n d) -> p n d", d=CH),
                    gout[:].rearrange("p (n d) -> p n d", d=CH),
                    wb, AX.mult)
                add_dep_helper(i_wm.ins, i_gth.ins, sync=True, reason="graw")

                # ---- combine via matmul: psum IS the dense layout ----
                pt = ppool.tile([P, 3 * S // 4], F32, tag="psum")
                for bq in range(16):
                    b, q = bq // 4, bq % 4
                    nc.tensor.matmul(
                        pt[:, :],
                        wsel[:, P * bq:P * (bq + 1)],
                        gout[:, 1536 * q + 384 * b:1536 * q + 384 * b + 384],
                        start=(bq == 0), stop=(bq == 15))

                # ---- post: clip, gamma, color, clip ----
                d_t = wpool.tile([P, 3 * S // 4], F32, tag="d_t")
                d_u = wpool.tile([P, S // 4], F32, tag="d_u")
                nc.vector.tensor_scalar(pt[:], pt[:], 1e-30, 1.0,
                                        AX.max, AX.min)
                nc.scalar.activation(d_t[:], pt[:], AF.Ln)
                nc.scalar.activation(d_t[:], d_t[:], AF.Exp,
                                     scale=cf[:, D_G:D_G + 1])
                d_c = wpool.tile([P, 3 * S // 4], F32, tag="d_c")
                nch = S // 4
                dv3 = d_t[:].rearrange("p (n c) -> p n c", c=CH)
                tv3 = d_c[:].rearrange("p (n c) -> p n c", c=CH)
                uv = d_u[:].rearrange("p (n o) -> p n o", o=1)
                for c3 in range(3):
                    nc.vector.tensor_scalar(
                        tv3[:, :, c3:c3 + 1], dv3[:, :, 0:1],
                        sc(D_M + 3 * c3), None, AX.mult)
                    nc.vector.tensor_scalar(
                        uv, dv3[:, :, 1:2], sc(D_M + 3 * c3 + 1), None,
                        AX.mult)
                    nc.vector.tensor_tensor(
                        tv3[:, :, c3:c3 + 1], tv3[:, :, c3:c3 + 1], uv,
                        AX.add)
                    nc.vector.tensor_scalar(
                        uv, dv3[:, :, 2:3], sc(D_M + 3 * c3 + 2), None,
                        AX.mult)
                    nc.vector.tensor_tensor(
                        tv3[:, :, c3:c3 + 1], tv3[:, :, c3:c3 + 1], uv,
                        AX.add)
                nc.vector.tensor_scalar(d_c[:], d_c[:], 0.0, 1.0,
                                        AX.max, AX.min)

                # ---- output stores ----
                imgl, strip = r // 16, r % 16
                for r2 in range(8):
                    off = imgl * IMGEL + (32 * strip + r2) * ROWB
                    dst = bass.AP(out_d, off,
                                  [[8 * ROWB, 4], [48, 32], [1, 48]])
                    e = nc.sync if r2 % 2 == 0 else nc.scalar
                    e.dma_start(dst, d_c[:, 48 * r2:48 * r2 + 48])
    nc.compile()
    return nc


_CACHE = {}


def kernel(images, geom_u, color_u, cutout_u):
    _install_profhook()
    images = np.asarray(images, dtype=np.float32)
    geom_u = np.asarray(geom_u, dtype=np.float32)
    color_u = np.asarray(color_u, dtype=np.float32)
    cutout_u = np.asarray(cutout_u, dtype=np.float32)
    Ms, Mc, gamma, capply, ccx, ccy, ccw, cch = host_params(
        geom_u, color_u, cutout_u)
    RB, CB = global_band_shape(Ms)
    nround = int(os.environ.get('KROUNDS', NROUND_FULL))
    sim_floor = os.environ.get("KSIM", "0") == "1"
    key = (RB, CB, nround, sim_floor)
    if key not in _CACHE:
        _CACHE[key] = build_kernel(RB, CB, nround, sim_floor=sim_floor)
    nc = _CACHE[key]
    wsel = build_wsel()
    in_maps = []
    for core in range(NCORES):
        dmaoff, coef = build_tables(Ms, Mc, gamma, capply, ccx, ccy, ccw,
                                    cch, core, nround, RB, CB, sim_floor)
        shard = images[core * IPC:(core + 1) * IPC].reshape(-1)
        pconst = (((np.arange(P) % 32) >= 16)
                  .astype(np.float32).reshape(P, 1))
        in_maps.append({"img": np.ascontiguousarray(shard),
                        "dmaoff": dmaoff, "coef": coef, "wsel": wsel,
                        "pconst": pconst})
    if sim_floor:
        from concourse.bass_interp import CoreSim
        sim = CoreSim(nc, trace=False)
        for k, v in in_maps[0].items():
            sim.tensor(k)[:] = v
        sim.simulate()
        out = np.zeros((B, H, W, CH), np.float32)
        out[0:IPC] = np.array(sim.tensor("out")).reshape(IPC, H, W, CH)
        return out
    trace = os.environ.get("KTRACE", "0") == "1"
    res = run_bass_kernel_spmd(nc, in_maps, core_ids=list(range(NCORES)),
                               trace=trace, trace_cores=[0] if trace else None)
    if trace and res.exec_time_ns:
        print(f"HW exec time: {res.exec_time_ns} ns")
    out = np.zeros((B, H, W, CH), np.float32)
    for core in range(NCORES):
        out[core * IPC:(core + 1) * IPC] = (
            res.results[core]["out"].reshape(IPC, H, W, CH))
    return out
